# revision 16
# baseline (speedup 1.0000x reference)
"""Multi-head self-attention (B=2, S=2048, D=1024, H=16) on 8 TRN2 NeuronCores.

Tensor-parallel over heads: each core owns 2 heads. Accepts FULL inputs,
returns FULL output. Host pre-transposes x and slices per-head weights;
each core computes qkv -> per-head LayerNorm -> attention -> partial
output projection (over its 128 embed dims); host sums the 8 partials
and adds the projection bias.

v2 layout of the per-core program:
  passA: per 128-token block: qkv matmuls -> grouped bn_stats -> raw
         q/k + v evicted to SBUF (bf16).
  passB: LN constants for all 32 blocks batched in ~8 wide vector ops
         (combine even/odd bn_stats pipes, one ACT sqrt, one DVE
         approx-reciprocal).
  passC: per block: 4 fused tensor_scalar LN applies (bf16 4x mode),
         2 PE transposes, one copy into the [dim, token] q/k store.
  phase2: software-pipelined attention: scores run 2 k-blocks ahead of
         the exp stream, A@V runs 2 behind, the softmax denominator is
         inverted with a DVE approx reciprocal (no ACT table switches
         ever - ACT does only the exp stream), and each chunk's
         projection is deferred one chunk to fill the PE bubble at the
         chunk boundary.
"""

import os
import sys

import numpy as np

for _p in ("/opt/trn_rl_repo", "/root/.axon_site/_ro/trn_rl_repo"):
    if os.path.isdir(_p) and _p not in sys.path:
        sys.path.insert(0, _p)
        break

import concourse.bass as bass  # noqa: E402
import concourse.bacc as bacc  # noqa: E402
import concourse.tile as tile  # noqa: E402
from concourse import mybir  # noqa: E402
from concourse.bass_utils import run_bass_kernel_spmd  # noqa: E402

F32 = mybir.dt.float32
F32R = mybir.dt.float32r
BF16 = mybir.dt.bfloat16
AF = mybir.ActivationFunctionType
ALU = mybir.AluOpType

NCORES = 8
D = 1024
H = 16
HD = 64
HPC = H // NCORES          # heads per core = 2
DPC = HPC * HD             # embed dims per core = 128
EPS = 1e-5


def _r(ap):
    return ap.bitcast(F32R)


def build_nc(B, S, affine):
    """Build the SPMD Bass program for one core (same program, 8 cores)."""
    T = B * S                      # total token columns
    NTB = T // 128                 # 128-token blocks (32)
    NCH = T // 512                 # 512-token chunks (8)
    QC = S // 512                  # q-chunks per batch (4)
    KB = S // 128                  # k-blocks per batch (16)
    KCH = D // 128                 # contraction chunks (8)
    SCALE = 1.0 / np.sqrt(HD)

    nc = bacc.Bacc(
        "TRN2",
        target_bir_lowering=False,
        debug=False,
        enable_asserts=True,
        num_devices=NCORES,
    )

    xT = nc.dram_tensor("xT", [D, T], BF16, kind="ExternalInput").ap()
    wq = nc.dram_tensor("wt_qkv", [D, 3 * DPC], BF16, kind="ExternalInput").ap()
    bq = nc.dram_tensor("b_qkv_s", [1, 3 * DPC], BF16, kind="ExternalInput").ap()
    wp = nc.dram_tensor("wt_proj", [DPC, D], BF16, kind="ExternalInput").ap()
    ones = nc.dram_tensor("c_ones", [1, 512], F32R, kind="ExternalInput").ap()
    vones = nc.dram_tensor(
        "c_vones", [128, HPC, NTB, 1], BF16, kind="ExternalInput"
    ).ap()
    onesb = nc.dram_tensor("c_onesb", [1, 512], BF16, kind="ExternalInput").ap()
    e2 = nc.dram_tensor("c_e2", [64, 128], F32R, kind="ExternalInput").ap()
    den0 = nc.dram_tensor("c_den0", [64, 512], F32R, kind="ExternalInput").ap()
    eye = nc.dram_tensor("c_eye", [128, 128], BF16, kind="ExternalInput").ap()
    if affine:
        gb = nc.dram_tensor("c_gb", [128, 4, HD], F32, kind="ExternalInput").ap()
    outp = nc.dram_tensor("outp", [T, D], BF16, kind="ExternalOutput").ap()

    from contextlib import ExitStack

    with tile.TileContext(nc) as tc, ExitStack() as stack:
        const = stack.enter_context(tc.tile_pool(name="const", bufs=1))
        persist = stack.enter_context(tc.tile_pool(name="persist", bufs=1))

        # weights first so the first qkv matmul can start asap
        wq_sb = const.tile([128, KCH, 3 * DPC], BF16, tag="wq")
        nc.sync.dma_start(
            out=wq_sb, in_=wq.rearrange("(c p) n -> p c n", p=128)
        )
        bq_sb = const.tile([1, 3 * DPC], BF16, tag="bq")
        nc.sync.dma_start(out=bq_sb, in_=bq)
        onesb_sb = const.tile([1, 512], BF16, tag="onesb")
        nc.sync.dma_start(out=onesb_sb, in_=onesb)
        eye_sb = const.tile([128, 128], BF16, tag="eye")
        nc.sync.dma_start(out=eye_sb, in_=eye)
        ones_sb = const.tile([1, 512], F32R, tag="ones")
        nc.sync.dma_start(out=ones_sb, in_=ones)
        e2_sb = const.tile([64, 128], F32R, tag="e2")
        nc.sync.dma_start(out=e2_sb, in_=e2)
        wp_sb = const.tile([DPC, D], BF16, tag="wp")
        nc.sync.dma_start(out=wp_sb, in_=wp)
        eps_sb = const.tile([128, 1], F32, tag="eps")
        nc.vector.memset(eps_sb, EPS)
        if affine:
            gb_sb = const.tile([128, 4, HD], F32, tag="gb")
            nc.sync.dma_start(out=gb_sb, in_=gb)

        # persistent intermediates
        qkT = persist.tile([128, 2, T], BF16, tag="qkT")   # plane0=q^T plane1=k^T
        vO = persist.tile([128, HPC, NTB, HD + 1], BF16, tag="vO")
        aT = persist.tile([128, T], BF16, tag="aT")        # attention out^T
        qkraw = persist.tile([128, NTB, 2, 2 * HD], BF16, tag="qkraw")
        st_all = persist.tile([128, NTB, 4, 6], F32, tag="st")
        mu_t = persist.tile([128, NTB, 4], F32, tag="mu")
        rstd_t = persist.tile([128, NTB, 4], F32, tag="rstd")
        w0 = persist.tile([128, NTB, 4], F32, tag="w0")    # scratch
        w1 = persist.tile([128, NTB, 4], F32, tag="w1")    # scratch
        den64 = persist.tile([64, 512], F32R, tag="den64")
        nc.sync.dma_start(out=den64, in_=den0)
        nc.sync.dma_start(out=vO[:, :, :, HD : HD + 1], in_=vones)

        # ---------------- Phase 1 passA: qkv + stats + raw eviction -----
        with (
            tc.tile_pool(name="xt", bufs=2) as xt_pool,
            tc.tile_pool(name="qkv_ps", bufs=4, space="PSUM") as qkv_ps,
        ):
            for n in range(NCH):
                xt = xt_pool.tile([128, KCH, 512], BF16, tag="xt")
                nc.sync.dma_start(
                    out=xt,
                    in_=xT.rearrange("(c p) t -> p c t", p=128)[
                        :, :, n * 512 : (n + 1) * 512
                    ],
                )
                for tbl in range(4):
                    tb = n * 4 + tbl
                    ps = qkv_ps.tile([128, 3 * DPC], F32, tag="ps")
                    nc.tensor.matmul(
                        ps,
                        lhsT=onesb_sb[0:1, 0:128],
                        rhs=bq_sb,
                        start=True,
                        stop=False,
                    )
                    for k in range(KCH):
                        nc.tensor.matmul(
                            ps,
                            lhsT=xt[:, k, tbl * 128 : (tbl + 1) * 128],
                            rhs=wq_sb[:, k, :],
                            start=False,
                            stop=(k == KCH - 1),
                        )
                    # per-group stats over the 4 (q/k, head) groups
                    for g in range(4):
                        nc.vector.bn_stats(
                            out=st_all[:, tb, g],
                            in_=ps[:, g * HD : (g + 1) * HD],
                        )
                    # raw q/k (bf16) for the deferred LN apply
                    nc.vector.tensor_copy(
                        out=qkraw[:, tb],
                        in_=ps[:, 0 : 2 * DPC].rearrange("p (c d) -> p c d", d=2 * HD),
                    )
                    # v straight into the attention-value store
                    nc.vector.tensor_copy(
                        out=vO[:, :, tb, 0:HD],
                        in_=ps[:, 2 * DPC :].rearrange("p (h d) -> p h d", d=HD),
                    )

        # ---------------- passB: batched LN constants --------------------
        # st layout per group: [cnt_e, mean_e, M2_e, cnt_o, mean_o, M2_o]
        me = st_all[:, :, :, 1]
        mo = st_all[:, :, :, 4]
        m2e = st_all[:, :, :, 2]
        m2o = st_all[:, :, :, 5]
        nc.vector.tensor_add(mu_t, me, mo)                   # 2*mu
        nc.vector.tensor_sub(w0, me, mo)                     # d
        nc.vector.tensor_add(w1, m2e, m2o)                   # M2e+M2o = 64*var_w
        nc.vector.tensor_mul(w0, w0, w0)                     # d^2
        # var = (M2e+M2o)/64 + d^2/4  ->  64*var = w1 + 16*d^2
        nc.vector.affine_then_add(out=w0, in0=w0, in1=w1, scale=16.0, bias=0.0)
        # rstd = 1/sqrt(var + eps)
        nc.scalar.activation(
            out=w1, in_=w0, func=AF.Sqrt, bias=eps_sb, scale=1.0 / HD
        )
        nc.vector.reciprocal_approx_fast(out=rstd_t, in_=w1)
        nc.vector.tensor_scalar(
            out=mu_t, in0=mu_t, scalar1=0.5, scalar2=None, op0=ALU.mult
        )

        # ---------------- passC: LN apply + transpose --------------------
        with (
            tc.tile_pool(name="qn", bufs=3) as qn_pool,
            tc.tile_pool(name="t_ps", bufs=3, space="PSUM") as t_ps,
        ):
            for tb in range(NTB):
                qn = qn_pool.tile([128, 2, 2 * HD], BF16, tag="qn")
                for g in range(4):
                    pl, hh = g // 2, g % 2
                    dsl = qn[:, pl, hh * HD : (hh + 1) * HD]
                    nc.vector.tensor_scalar(
                        out=dsl,
                        in0=qkraw[:, tb, pl, hh * HD : (hh + 1) * HD],
                        scalar1=mu_t[:, tb, g : g + 1],
                        scalar2=rstd_t[:, tb, g : g + 1],
                        op0=ALU.subtract,
                        op1=ALU.mult,
                    )
                    if affine:
                        nc.vector.tensor_mul(dsl, dsl, gb_sb[:, 2 * pl, :])
                        nc.vector.tensor_add(dsl, dsl, gb_sb[:, 2 * pl + 1, :])
                tp = t_ps.tile([128, 2, 128], BF16, tag="tp")
                nc.tensor.transpose(tp[:, 0, :], qn[:, 0, :], eye_sb)
                nc.tensor.transpose(tp[:, 1, :], qn[:, 1, :], eye_sb)
                nc.vector.tensor_copy(
                    out=qkT[:, :, tb * 128 : (tb + 1) * 128], in_=tp
                )

        # ---------------- Phase 2: attention -----------------------------
        with (
            tc.tile_pool(name="sc_ps", bufs=2, space="PSUM") as sc_ps,
            tc.tile_pool(name="o_ps", bufs=1, space="PSUM") as o_ps,
            tc.tile_pool(name="sm_ps", bufs=2, space="PSUM") as sm_ps,
            tc.tile_pool(name="exps", bufs=3) as exps,
            tc.tile_pool(name="stage2", bufs=2) as stage2,
            tc.tile_pool(name="ostage", bufs=2) as ostage,
        ):
            seq = [(ci, kb) for ci in range(B * QC) for kb in range(KB)]
            ooms = {}
            exts = {}

            def emit_scores_exp(ci, kb):
                b, qc = divmod(ci, QC)
                cols = slice(b * S + qc * 512, b * S + (qc + 1) * 512)
                gkb = b * KB + kb
                ks = slice(gkb * 128, (gkb + 1) * 128)
                scp = sc_ps.tile([128, HPC, 512], F32, tag="s", name="scp")
                for h in range(HPC):
                    hp = slice(h * HD, (h + 1) * HD)
                    nc.tensor.matmul(
                        scp[:, h, :],
                        lhsT=qkT[hp, 1, ks],
                        rhs=qkT[hp, 0, cols],
                        start=True,
                        stop=True,
                    )
                ex = exps.tile([128, HPC, 512], BF16, tag="ex", name="ex")
                nc.scalar.activation(out=ex, in_=scp, func=AF.Exp, scale=SCALE)
                exts[(ci, kb)] = ex

            def emit_av(ci, kb):
                b, _ = divmod(ci, QC)
                gkb = b * KB + kb
                if kb == 0:
                    ooms[ci] = o_ps.tile(
                        [HD + 1, HPC, 512], F32, tag="o", name="oom"
                    )
                oom = ooms[ci]
                ex = exts.pop((ci, kb))
                for h in range(HPC):
                    nc.tensor.matmul(
                        oom[:, h, :],
                        lhsT=vO[:, h, gkb, :],
                        rhs=ex[:, h, :],
                        start=(kb == 0),
                        stop=(kb == KB - 1),
                    )

            def emit_tail(ci):
                b, qc = divmod(ci, QC)
                cols = slice(b * S + qc * 512, b * S + (qc + 1) * 512)
                oom = ooms.pop(ci)
                for h in range(HPC):
                    nc.vector.tensor_copy(
                        out=den64[32 * h : 32 * h + 1, :],
                        in_=oom[HD : HD + 1, h, :],
                    )
                rb = sm_ps.tile([128, 512], F32, tag="sm", name="rb")
                nc.tensor.matmul(rb, lhsT=e2_sb, rhs=den64, start=True, stop=True)
                rbs = stage2.tile([128, 512], F32, tag="rbs", name="rbs")
                nc.vector.reciprocal_approx_fast(out=rbs, in_=rb)
                for h in range(HPC):
                    nc.vector.tensor_mul(
                        aT[h * HD : (h + 1) * HD, cols],
                        oom[0:HD, h, :],
                        rbs[h * HD : (h + 1) * HD, :],
                    )

            def emit_proj(ci):
                for tbl in range(4):
                    tb = ci * 4 + tbl
                    ob = ostage.tile([128, D], BF16, tag="ob")
                    for nn in range(D // 512):
                        pps = sm_ps.tile([128, 512], F32, tag="sm", name="pps")
                        nc.tensor.matmul(
                            pps,
                            lhsT=aT[:, tb * 128 : (tb + 1) * 128],
                            rhs=wp_sb[:, nn * 512 : (nn + 1) * 512],
                            start=True,
                            stop=True,
                        )
                        nc.vector.tensor_copy(
                            out=ob[:, nn * 512 : (nn + 1) * 512], in_=pps
                        )
                    nc.sync.dma_start(
                        out=outp[tb * 128 : (tb + 1) * 128, :], in_=ob
                    )

            for idx, (ci, kb) in enumerate(seq):
                emit_scores_exp(ci, kb)
                if idx >= 2:
                    emit_av(*seq[idx - 2])
                if kb == 1 and ci >= 1:
                    emit_tail(ci - 1)
                    if ci >= 2:
                        emit_proj(ci - 2)
            emit_av(*seq[-2])
            emit_av(*seq[-1])
            last = B * QC - 1
            emit_proj(last - 1)
            emit_tail(last)
            emit_proj(last)

    nc.compile()
    return nc


def make_in_maps(x, w_qkv, b_qkv, w_proj, q_gamma, q_beta, k_gamma, k_beta,
                 affine):
    import ml_dtypes

    bf = ml_dtypes.bfloat16
    B, S, _ = x.shape
    T = B * S
    xT = np.ascontiguousarray(x.reshape(T, D).T).astype(bf)
    ones = np.ones((1, 512), np.float32)
    onesb = np.ones((1, 512), bf)
    vones = np.ones((128, HPC, (T // 128), 1), bf)
    eye = np.eye(128, dtype=np.float32).astype(bf)
    in_maps = []
    for c in range(NCORES):
        rs = slice(c * DPC, (c + 1) * DPC)
        w_slice = np.concatenate(
            [w_qkv[rs], w_qkv[D:2 * D][rs.start:rs.stop],
             w_qkv[2 * D:][rs.start:rs.stop]],
            axis=0,
        )  # [384, 1024]
        b_slice = np.concatenate(
            [b_qkv[rs], b_qkv[D:2 * D][rs.start:rs.stop],
             b_qkv[2 * D:][rs.start:rs.stop]]
        )[None, :]  # [1, 384]
        e2 = np.zeros((64, 128), np.float32)
        e2[0, 0:HD] = 1.0
        e2[32, HD:128] = 1.0
        m = {
            "xT": xT,
            "wt_qkv": np.ascontiguousarray(w_slice.T).astype(bf),
            "b_qkv_s": np.ascontiguousarray(b_slice).astype(bf),
            "wt_proj": np.ascontiguousarray(w_proj[:, rs].T).astype(bf),
            "c_ones": ones,
            "c_vones": vones,
            "c_onesb": onesb,
            "c_eye": eye,
            "c_e2": e2,
            "c_den0": np.zeros((64, 512), np.float32),
        }
        if affine:
            gbs = np.stack([q_gamma, q_beta, k_gamma, k_beta])  # [4, 64]
            m["c_gb"] = np.ascontiguousarray(
                np.broadcast_to(gbs[None], (128, 4, HD)).astype(np.float32)
            )
        in_maps.append(m)
    return in_maps


_NC_CACHE = {}

LAST_RESULTS = None


def kernel(x, w_qkv, b_qkv, w_proj, b_proj, q_gamma, q_beta, k_gamma, k_beta,
           **unused):
    global LAST_RESULTS
    x = np.asarray(x, np.float32)
    w_qkv = np.asarray(w_qkv, np.float32)
    b_qkv = np.asarray(b_qkv, np.float32)
    w_proj = np.asarray(w_proj, np.float32)
    b_proj = np.asarray(b_proj, np.float32)
    q_gamma = np.asarray(q_gamma, np.float32)
    q_beta = np.asarray(q_beta, np.float32)
    k_gamma = np.asarray(k_gamma, np.float32)
    k_beta = np.asarray(k_beta, np.float32)

    B, S, _ = x.shape
    affine = not (
        np.all(q_gamma == 1) and np.all(k_gamma == 1)
        and np.all(q_beta == 0) and np.all(k_beta == 0)
    )
    key = (B, S, affine)
    if key not in _NC_CACHE:
        _NC_CACHE[key] = build_nc(B, S, affine)
    nc = _NC_CACHE[key]

    in_maps = make_in_maps(
        x, w_qkv, b_qkv, w_proj, q_gamma, q_beta, k_gamma, k_beta, affine
    )
    trace = bool(int(os.environ.get("BASS_KERNEL_TRACE", "0")))
    res = run_bass_kernel_spmd(
        nc, in_maps, core_ids=list(range(NCORES)), trace=trace
    )
    LAST_RESULTS = res
    acc = np.zeros((B * S, D), np.float32)
    for r in res.results:
        acc += r["outp"].astype(np.float32)
    acc += b_proj[None, :]
    return acc.reshape(B, S, D)


# revision 20
# speedup vs baseline: 1.2960x; 1.2960x over previous
"""Multi-head self-attention (B=2, S=2048, D=1024, H=16) on 8 TRN2 NeuronCores.

Tensor-parallel over heads: each core owns 2 heads. Accepts FULL inputs,
returns FULL output. Host pre-transposes x and slices per-head weights;
each core computes qkv -> per-head LayerNorm -> attention -> partial
output projection (over its 128 embed dims); host sums the 8 partials
and adds the projection bias.

v2 layout of the per-core program:
  passA: per 128-token block: qkv matmuls -> grouped bn_stats -> raw
         q/k + v evicted to SBUF (bf16).
  passB: LN constants for all 32 blocks batched in ~8 wide vector ops
         (combine even/odd bn_stats pipes, one ACT sqrt, one DVE
         approx-reciprocal).
  passC: per block: 4 fused tensor_scalar LN applies (bf16 4x mode),
         2 PE transposes, one copy into the [dim, token] q/k store.
  phase2: software-pipelined attention: scores run 2 k-blocks ahead of
         the exp stream, A@V runs 2 behind, the softmax denominator is
         inverted with a DVE approx reciprocal (no ACT table switches
         ever - ACT does only the exp stream), and each chunk's
         projection is deferred one chunk to fill the PE bubble at the
         chunk boundary.
"""

import os
import sys

import numpy as np

for _p in ("/opt/trn_rl_repo", "/root/.axon_site/_ro/trn_rl_repo"):
    if os.path.isdir(_p) and _p not in sys.path:
        sys.path.insert(0, _p)
        break

import concourse.bass as bass  # noqa: E402
import concourse.bacc as bacc  # noqa: E402
import concourse.tile as tile  # noqa: E402
from concourse import mybir  # noqa: E402
from concourse.bass_utils import run_bass_kernel_spmd  # noqa: E402

F32 = mybir.dt.float32
F32R = mybir.dt.float32r
BF16 = mybir.dt.bfloat16
AF = mybir.ActivationFunctionType
ALU = mybir.AluOpType

NCORES = 8
D = 1024
H = 16
HD = 64
HPC = H // NCORES          # heads per core = 2
DPC = HPC * HD             # embed dims per core = 128
EPS = 1e-5


def _r(ap):
    return ap.bitcast(F32R)


def build_nc(B, S, affine):
    """Build the SPMD Bass program for one core (same program, 8 cores)."""
    T = B * S                      # total token columns
    NTB = T // 128                 # 128-token blocks (32)
    NCH = T // 512                 # 512-token chunks (8)
    QC = S // 512                  # q-chunks per batch (4)
    KB = S // 128                  # k-blocks per batch (16)
    KCH = D // 128                 # contraction chunks (8)
    SCALE = 1.0 / np.sqrt(HD)

    nc = bacc.Bacc(
        "TRN2",
        target_bir_lowering=False,
        debug=False,
        enable_asserts=True,
        num_devices=NCORES,
    )

    xT = nc.dram_tensor("xT", [D, T], BF16, kind="ExternalInput").ap()
    wq = nc.dram_tensor("wt_qkv", [D, 3 * DPC], BF16, kind="ExternalInput").ap()
    bq = nc.dram_tensor("b_qkv_s", [1, 3 * DPC], BF16, kind="ExternalInput").ap()
    wp = nc.dram_tensor("wt_proj", [DPC, D], BF16, kind="ExternalInput").ap()
    ones = nc.dram_tensor("c_ones", [1, 512], F32R, kind="ExternalInput").ap()
    vones = nc.dram_tensor(
        "c_vones", [128, HPC, NTB, 1], BF16, kind="ExternalInput"
    ).ap()
    onesb = nc.dram_tensor("c_onesb", [1, 512], BF16, kind="ExternalInput").ap()
    e2 = nc.dram_tensor("c_e2", [64, 128], F32R, kind="ExternalInput").ap()
    den0 = nc.dram_tensor("c_den0", [64, 512], F32R, kind="ExternalInput").ap()
    eye = nc.dram_tensor("c_eye", [128, 128], BF16, kind="ExternalInput").ap()
    if affine:
        gb = nc.dram_tensor("c_gb", [128, 4, HD], F32, kind="ExternalInput").ap()
    outp = nc.dram_tensor("outp", [T, D], BF16, kind="ExternalOutput").ap()

    from contextlib import ExitStack

    with tile.TileContext(nc) as tc, ExitStack() as stack:
        const = stack.enter_context(tc.tile_pool(name="const", bufs=1))
        persist = stack.enter_context(tc.tile_pool(name="persist", bufs=1))

        # weights needed by passA go out first on the sync queue; the
        # late-use constants ride the gpsimd (SWDGE) queue so the first
        # x chunk isn't stuck behind them.
        wq_sb = const.tile([128, KCH, 3 * DPC], BF16, tag="wq")
        nc.sync.dma_start(
            out=wq_sb, in_=wq.rearrange("(c p) n -> p c n", p=128)
        )
        bq_sb = const.tile([1, 3 * DPC], BF16, tag="bq")
        nc.sync.dma_start(out=bq_sb, in_=bq)
        onesb_sb = const.tile([1, 512], BF16, tag="onesb")
        nc.sync.dma_start(out=onesb_sb, in_=onesb)
        eye_sb = const.tile([128, 128], BF16, tag="eye")
        nc.gpsimd.dma_start(out=eye_sb, in_=eye)
        ones_sb = const.tile([1, 512], F32R, tag="ones")
        nc.gpsimd.dma_start(out=ones_sb, in_=ones)
        e2_sb = const.tile([64, 128], F32R, tag="e2")
        nc.gpsimd.dma_start(out=e2_sb, in_=e2)
        wp_sb = const.tile([DPC, D], BF16, tag="wp")
        nc.gpsimd.dma_start(out=wp_sb, in_=wp)
        eps_sb = const.tile([128, 1], F32, tag="eps")
        nc.vector.memset(eps_sb, EPS)
        if affine:
            gb_sb = const.tile([128, 4, HD], F32, tag="gb")
            nc.gpsimd.dma_start(out=gb_sb, in_=gb)

        # persistent intermediates
        qkT = persist.tile([128, 2, T], BF16, tag="qkT")   # plane0=q^T plane1=k^T
        vO = persist.tile([128, HPC, NTB, HD + 1], BF16, tag="vO")
        aT = persist.tile([128, T], BF16, tag="aT")        # attention out^T
        den64 = [
            persist.tile([64, 512], F32R, tag=f"den64{i}", name=f"den64{i}")
            for i in range(2)
        ]
        nc.gpsimd.dma_start(out=den64[0], in_=den0)
        nc.gpsimd.dma_start(out=den64[1], in_=den0)
        nc.gpsimd.dma_start(out=vO[:, :, :, HD : HD + 1], in_=vones)

        # ------- Phase 1: per-block qkv + LN + transpose pipeline --------
        # Work spread over three engines so no single one binds:
        #   PE:  bias+qkv matmuls, 2 transposes
        #   DVE: bn_stats/aggr, rstd recip, -mu*rstd
        #   ACT: sqrt, the 4 LN applies (scale/bias APs), v cast, qkT evict
        with (
            tc.tile_pool(name="xt", bufs=2) as xt_pool,
            tc.tile_pool(name="qkv_ps", bufs=4, space="PSUM") as qkv_ps,
            tc.tile_pool(name="stats", bufs=4) as stats_pool,
            tc.tile_pool(name="qn", bufs=3) as qn_pool,
            tc.tile_pool(name="t_ps", bufs=3, space="PSUM") as t_ps,
        ):
            for n in range(NCH):
                xt = xt_pool.tile([128, KCH, 512], BF16, tag="xt")
                nc.sync.dma_start(
                    out=xt,
                    in_=xT.rearrange("(c p) t -> p c t", p=128)[
                        :, :, n * 512 : (n + 1) * 512
                    ],
                )
                for tbl in range(4):
                    tb = n * 4 + tbl
                    ps = qkv_ps.tile([128, 3 * DPC], F32, tag="ps")
                    nc.tensor.matmul(
                        ps,
                        lhsT=onesb_sb[0:1, 0:128],
                        rhs=bq_sb,
                        start=True,
                        stop=False,
                    )
                    for k in range(KCH):
                        nc.tensor.matmul(
                            ps,
                            lhsT=xt[:, k, tbl * 128 : (tbl + 1) * 128],
                            rhs=wq_sb[:, k, :],
                            start=False,
                            stop=(k == KCH - 1),
                        )
                    st = stats_pool.tile([128, 4, 6], F32, tag="st")
                    mv = stats_pool.tile([128, 4, 2], F32, tag="mv")
                    for g in range(4):
                        nc.vector.bn_stats(
                            out=st[:, g], in_=ps[:, g * HD : (g + 1) * HD]
                        )
                        nc.vector.bn_aggr(out=mv[:, g], in_=st[:, g])
                    sd = stats_pool.tile([128, 4], F32, tag="sd")
                    nc.scalar.activation(
                        out=sd, in_=mv[:, :, 1], func=AF.Sqrt, bias=eps_sb
                    )
                    rstd = stats_pool.tile([128, 4], F32, tag="rstd")
                    nc.vector.reciprocal(out=rstd, in_=sd)
                    negmu = stats_pool.tile([128, 4], F32, tag="negmu")
                    nc.vector.tensor_scalar(
                        out=negmu, in0=mv[:, :, 0], scalar1=-1.0,
                        scalar2=None, op0=ALU.mult,
                    )
                    nmr = stats_pool.tile([128, 4], F32, tag="nmr")
                    nc.vector.tensor_mul(nmr, negmu, rstd)
                    qn = qn_pool.tile([128, 2, 2 * HD], BF16, tag="qn")
                    for g in range(4):
                        pl, hh = g // 2, g % 2
                        dsl = qn[:, pl, hh * HD : (hh + 1) * HD]
                        nc.scalar.activation(
                            out=dsl,
                            in_=ps[:, g * HD : (g + 1) * HD],
                            func=AF.Identity,
                            bias=nmr[:, g : g + 1],
                            scale=rstd[:, g : g + 1],
                        )
                        if affine:
                            nc.vector.tensor_mul(dsl, dsl, gb_sb[:, 2 * pl, :])
                            nc.vector.tensor_add(dsl, dsl, gb_sb[:, 2 * pl + 1, :])
                    nc.scalar.copy(
                        out=vO[:, :, tb, 0:HD],
                        in_=ps[:, 2 * DPC :].rearrange("p (h d) -> p h d", d=HD),
                    )
                    tp = t_ps.tile([128, 256], BF16, tag="tp")
                    nc.tensor.transpose(tp[:, 0:128], qn[:, 0, :], eye_sb)
                    nc.tensor.transpose(tp[:, 128:256], qn[:, 1, :], eye_sb)
                    nc.scalar.copy(
                        out=qkT[:, :, tb * 128 : (tb + 1) * 128],
                        in_=tp.rearrange("p (c d) -> p c d", d=128),
                    )

        # ---------------- Phase 2: attention -----------------------------
        with (
            tc.tile_pool(name="sc_ps", bufs=2, space="PSUM") as sc_ps,
            tc.tile_pool(name="o_ps", bufs=1, space="PSUM") as o_ps,
            tc.tile_pool(name="sm_ps", bufs=2, space="PSUM") as sm_ps,
            tc.tile_pool(name="exps", bufs=4) as exps,
            tc.tile_pool(name="stage2", bufs=2) as stage2,
            tc.tile_pool(name="ostage", bufs=2) as ostage,
        ):
            seq = [(ci, kb) for ci in range(B * QC) for kb in range(KB)]
            ooms = {}
            exts = {}

            def emit_scores_exp(ci, kb):
                b, qc = divmod(ci, QC)
                cols = slice(b * S + qc * 512, b * S + (qc + 1) * 512)
                gkb = b * KB + kb
                ks = slice(gkb * 128, (gkb + 1) * 128)
                scp = sc_ps.tile([128, HPC, 512], F32, tag="s", name="scp")
                for h in range(HPC):
                    hp = slice(h * HD, (h + 1) * HD)
                    nc.tensor.matmul(
                        scp[:, h, :],
                        lhsT=qkT[hp, 1, ks],
                        rhs=qkT[hp, 0, cols],
                        start=True,
                        stop=True,
                    )
                ex = exps.tile([128, HPC, 512], BF16, tag="ex", name="ex")
                nc.scalar.activation(out=ex, in_=scp, func=AF.Exp, scale=SCALE)
                exts[(ci, kb)] = ex

            def emit_av(ci, kb):
                b, _ = divmod(ci, QC)
                gkb = b * KB + kb
                if kb == 0:
                    ooms[ci] = o_ps.tile(
                        [HD + 1, HPC, 512], F32, tag="o", name="oom"
                    )
                oom = ooms[ci]
                ex = exts.pop((ci, kb))
                for h in range(HPC):
                    nc.tensor.matmul(
                        oom[:, h, :],
                        lhsT=vO[:, h, gkb, :],
                        rhs=ex[:, h, :],
                        start=(kb == 0),
                        stop=(kb == KB - 1),
                    )

            def emit_tail(ci):
                b, qc = divmod(ci, QC)
                cols = slice(b * S + qc * 512, b * S + (qc + 1) * 512)
                oom = ooms.pop(ci)
                dn = den64[ci % 2]
                for h in range(HPC):
                    nc.vector.tensor_copy(
                        out=dn[32 * h : 32 * h + 1, :],
                        in_=oom[HD : HD + 1, h, :],
                    )
                rb = sm_ps.tile([128, 512], F32, tag="sm", name="rb")
                nc.tensor.matmul(rb, lhsT=e2_sb, rhs=dn, start=True, stop=True)
                rbs = stage2.tile([128, 512], F32, tag="rbs", name="rbs")
                nc.vector.reciprocal_approx_fast(out=rbs, in_=rb)
                for h in range(HPC):
                    nc.vector.tensor_mul(
                        aT[h * HD : (h + 1) * HD, cols],
                        oom[0:HD, h, :],
                        rbs[h * HD : (h + 1) * HD, :],
                    )

            def emit_proj(ci):
                for tbl in range(4):
                    tb = ci * 4 + tbl
                    ob = ostage.tile([128, D], BF16, tag="ob")
                    for nn in range(D // 512):
                        pps = sm_ps.tile([128, 512], F32, tag="sm", name="pps")
                        nc.tensor.matmul(
                            pps,
                            lhsT=aT[:, tb * 128 : (tb + 1) * 128],
                            rhs=wp_sb[:, nn * 512 : (nn + 1) * 512],
                            start=True,
                            stop=True,
                        )
                        nc.vector.tensor_copy(
                            out=ob[:, nn * 512 : (nn + 1) * 512], in_=pps
                        )
                    nc.sync.dma_start(
                        out=outp[tb * 128 : (tb + 1) * 128, :], in_=ob
                    )

            for idx, (ci, kb) in enumerate(seq):
                emit_scores_exp(ci, kb)
                if idx >= 2:
                    emit_av(*seq[idx - 2])
                if kb == 1 and ci >= 1:
                    emit_tail(ci - 1)
                    if ci >= 2:
                        emit_proj(ci - 2)
            emit_av(*seq[-2])
            emit_av(*seq[-1])
            last = B * QC - 1
            emit_proj(last - 1)
            emit_tail(last)
            emit_proj(last)

    nc.compile()
    return nc


def make_in_maps(x, w_qkv, b_qkv, w_proj, q_gamma, q_beta, k_gamma, k_beta,
                 affine):
    import ml_dtypes

    bf = ml_dtypes.bfloat16
    B, S, _ = x.shape
    T = B * S
    xT = np.ascontiguousarray(x.reshape(T, D).T).astype(bf)
    ones = np.ones((1, 512), np.float32)
    onesb = np.ones((1, 512), bf)
    vones = np.ones((128, HPC, (T // 128), 1), bf)
    eye = np.eye(128, dtype=np.float32).astype(bf)
    in_maps = []
    for c in range(NCORES):
        rs = slice(c * DPC, (c + 1) * DPC)
        w_slice = np.concatenate(
            [w_qkv[rs], w_qkv[D:2 * D][rs.start:rs.stop],
             w_qkv[2 * D:][rs.start:rs.stop]],
            axis=0,
        )  # [384, 1024]
        b_slice = np.concatenate(
            [b_qkv[rs], b_qkv[D:2 * D][rs.start:rs.stop],
             b_qkv[2 * D:][rs.start:rs.stop]]
        )[None, :]  # [1, 384]
        e2 = np.zeros((64, 128), np.float32)
        e2[0, 0:HD] = 1.0
        e2[32, HD:128] = 1.0
        m = {
            "xT": xT,
            "wt_qkv": np.ascontiguousarray(w_slice.T).astype(bf),
            "b_qkv_s": np.ascontiguousarray(b_slice).astype(bf),
            "wt_proj": np.ascontiguousarray(w_proj[:, rs].T).astype(bf),
            "c_ones": ones,
            "c_vones": vones,
            "c_onesb": onesb,
            "c_eye": eye,
            "c_e2": e2,
            "c_den0": np.zeros((64, 512), np.float32),
        }
        if affine:
            gbs = np.stack([q_gamma, q_beta, k_gamma, k_beta])  # [4, 64]
            m["c_gb"] = np.ascontiguousarray(
                np.broadcast_to(gbs[None], (128, 4, HD)).astype(np.float32)
            )
        in_maps.append(m)
    return in_maps


_NC_CACHE = {}

LAST_RESULTS = None


def kernel(x, w_qkv, b_qkv, w_proj, b_proj, q_gamma, q_beta, k_gamma, k_beta,
           **unused):
    global LAST_RESULTS
    x = np.asarray(x, np.float32)
    w_qkv = np.asarray(w_qkv, np.float32)
    b_qkv = np.asarray(b_qkv, np.float32)
    w_proj = np.asarray(w_proj, np.float32)
    b_proj = np.asarray(b_proj, np.float32)
    q_gamma = np.asarray(q_gamma, np.float32)
    q_beta = np.asarray(q_beta, np.float32)
    k_gamma = np.asarray(k_gamma, np.float32)
    k_beta = np.asarray(k_beta, np.float32)

    B, S, _ = x.shape
    affine = not (
        np.all(q_gamma == 1) and np.all(k_gamma == 1)
        and np.all(q_beta == 0) and np.all(k_beta == 0)
    )
    key = (B, S, affine)
    if key not in _NC_CACHE:
        _NC_CACHE[key] = build_nc(B, S, affine)
    nc = _NC_CACHE[key]

    in_maps = make_in_maps(
        x, w_qkv, b_qkv, w_proj, q_gamma, q_beta, k_gamma, k_beta, affine
    )
    trace = bool(int(os.environ.get("BASS_KERNEL_TRACE", "0")))
    res = run_bass_kernel_spmd(
        nc, in_maps, core_ids=list(range(NCORES)), trace=trace
    )
    LAST_RESULTS = res
    acc = np.zeros((B * S, D), np.float32)
    for r in res.results:
        acc += r["outp"].astype(np.float32)
    acc += b_proj[None, :]
    return acc.reshape(B, S, D)
